# revision 1
# baseline (speedup 1.0000x reference)
"""Trainium2 Bass kernel for AttentionUpscaling (sparse attention rescoring).

Math (reference):
  hf_flat[b,n,:]  = hr_hf_patches[b,:,h,w]    (n = h*nw + w)   -- (B,N,D) D=1024
  base_flat       = same for base_hf_patches
  key_emb = pool+linear(hf)  = hf_flat @ Weff_k + bk           -- (B,N,E) E=128
  q_emb   = base_flat @ Weff_q + bq        (Weff = A_pool^T @ W, pooling is linear)
  prior, idx = top16(hr_attn[b,n,:])
  pair MLP: h = gelu(q@W1q + k@W1k + (q-k)@W1d + (q*k)@W1p + prior*w1p + b1)
          = gelu(q@(W1q+W1d) + k@(W1k-W1d) + (q*k)@W1p + prior*w1p + b1)
  resid = h@W2 + b2 ;  w = softmax(log(max(prior,1e-8)) + resid)
  out[b,n,:] = sum_k w_k * hf_flat[b, idx_k, :]

Sharding: queries (N) split across 8 cores; key tables all-gathered (kcat) /
host-replicated (hf16). Per core per batch: NQ=512 queries, PAIRS=8192.
"""

import os
import sys
import math
import numpy as np

sys.path.insert(0, "/opt/trn_rl_repo")

try:  # make the NTFF profile hook shim importable as antenv.axon_hooks
    import antenv

    _p = "/opt/trn_rl_repo/antenv"
    if os.path.isdir(_p) and _p not in list(antenv.__path__):
        antenv.__path__.append(_p)
except Exception:
    pass

import concourse.bass as bass
import concourse.bacc as bacc
import concourse.hw_specs as hw_specs

# The SWDGE Q7 gather kernels cost ~4-6us each on silicon (idx unwrap +
# descriptor gen), far above the stock model (~1.2us). Feed the Tile
# scheduler realistic numbers so the static schedule overlaps them.
hw_specs.TRN2Spec.SWDGE_FIXED_OVERHEAD_NS = 4500
hw_specs.TRN2Spec.SWDGE_NS_PER_DESCRIPTOR = 1.2
import concourse.mybir as mybir
import concourse.tile as tile
from concourse.bass_utils import run_bass_kernel_spmd

dt = mybir.dt
AF = mybir.ActivationFunctionType
ALU = mybir.AluOpType

STEM_C = 16
POOL = 4
P = 8


class Cfg:
    def __init__(self, nq=512, nk=4096, ncores=8):
        self.B = 2
        self.D = 1024
        self.E = 128
        self.H = 64
        self.K = 16
        self.din = STEM_C * POOL * POOL  # 256
        self.ncores = ncores
        self.nq = nq            # queries per core per batch
        self.nk = nk            # total keys (= N)
        self.pairs = nq * self.K
        self.nt = nq // 128     # topk tiles per batch
        self.nhalf = 2          # kpack gather halves
        self.ph = self.pairs // 2
        self.ncc = self.pairs // 512    # MLP chunks (512 pairs)
        self.ng = self.pairs // 1024    # weighted-sum gather chunks (1024 pairs)
        assert nq % 128 == 0 and self.ph % 128 == 0 and self.pairs % 1024 == 0


def build_nc(cfg: Cfg, debug=False, dbg=False):
    STAGE = int(os.environ.get("KSTAGE", "5"))
    HW_GELU = os.environ.get("KGELU", "hw") == "hw"
    B, D, E, H, K = cfg.B, cfg.D, cfg.E, cfg.H, cfg.K
    NQ, NK, PAIRS = cfg.nq, cfg.nk, cfg.pairs
    f32, f16, u16, i16 = dt.float32, dt.float16, dt.uint16, dt.int16

    nc = bacc.Bacc("TRN2", target_bir_lowering=False, debug=debug,
                   num_devices=cfg.ncores)

    # ---------------- DRAM parameters ----------------
    attn = nc.dram_tensor("attn", [B, NQ, NK], f32, kind="ExternalInput").ap()
    base_dm = nc.dram_tensor("base_dm16", [B, D, NQ], f16, kind="ExternalInput").ap()
    hfk_dm = nc.dram_tensor("hf_dm16", [B, D, NK], f16, kind="ExternalInput").ap()
    hf16 = nc.dram_tensor("hf16", [B, NK, D], f16, kind="ExternalInput").ap()
    wq_d = nc.dram_tensor("wq", [cfg.din, E], f32, kind="ExternalInput").ap()
    wk_d = nc.dram_tensor("wk", [cfg.din, E], f32, kind="ExternalInput").ap()
    w1_d = nc.dram_tensor("w1", [4 * E + 1, H], f32, kind="ExternalInput").ap()
    w2_d = nc.dram_tensor("w2", [H, 1], f32, kind="ExternalInput").ap()
    bq_d = nc.dram_tensor("bq", [E, 1], f32, kind="ExternalInput").ap()
    bk_d = nc.dram_tensor("bk", [E, 1], f32, kind="ExternalInput").ap()
    b1_d = nc.dram_tensor("b1", [H, 1], f32, kind="ExternalInput").ap()
    b2_d = nc.dram_tensor("b2", [1, 1], f32, kind="ExternalInput").ap()
    apool_d = nc.dram_tensor("apool", [cfg.din, D], f32, kind="ExternalInput").ap()
    mask_d = nc.dram_tensor("maskblk", [128, 8, 64], f32, kind="ExternalInput").ap()
    ident_d = nc.dram_tensor("ident16", [128, 128], f16, kind="ExternalInput").ap()
    out_d = nc.dram_tensor("out", [B, NQ, D], f32, kind="ExternalOutput").ap()
    if dbg:
        dbg_qT = nc.dram_tensor("dbg_qT", [E, NQ], f16, kind="ExternalOutput").ap()
        dbg_kT = nc.dram_tensor("dbg_kT", [E, NQ], f16, kind="ExternalOutput").ap()
        dbg_kcat = nc.dram_tensor("dbg_kcat", [NK, E], f16, kind="ExternalOutput").ap()
        dbg_idx = nc.dram_tensor("dbg_idx", [128, NQ // 128, K], dt.uint16, kind="ExternalOutput").ap()
        dbg_prior = nc.dram_tensor("dbg_prior", [128, NQ // 128, K], f32, kind="ExternalOutput").ap()
        dbg_idxp = nc.dram_tensor("dbg_idxp", [128, NQ], dt.uint16, kind="ExternalOutput").ap()
        dbg_kpack = nc.dram_tensor("dbg_kpack", [128, 1, 1024], f16, kind="ExternalOutput").ap()
        dbg_resid = nc.dram_tensor("dbg_resid", [PAIRS], f32, kind="ExternalOutput").ap()
        dbg_wn = nc.dram_tensor("dbg_wn", [PAIRS], f32, kind="ExternalOutput").ap()
        dbg_khf = nc.dram_tensor("dbg_khf", [128, 8, D], f16, kind="ExternalOutput").ap()
        dbg_wblk = nc.dram_tensor("dbg_wblk", [128, PAIRS // 128, 64], f16, kind="ExternalOutput").ap()

    with tile.TileContext(nc) as tc:
        with (
            tc.tile_pool(name="const", bufs=1) as constp,
            tc.tile_pool(name="dram", bufs=1, space="DRAM") as dramp,
            tc.tile_pool(name="psA", bufs=2, space="PSUM") as psA,
            tc.tile_pool(name="psB", bufs=2, space="PSUM") as psB,
            tc.tile_pool(name="psO", bufs=2, space="PSUM") as psO,
        ):
            # ================= init: weights =================
            initp = tc.alloc_tile_pool(name="init", bufs=1)
            wq_sb = initp.tile([128, 2, E], f32)
            wk_sb = initp.tile([128, 2, E], f32)
            nc.sync.dma_start(wq_sb[:], wq_d.rearrange("(c p) e -> p c e", p=128))
            nc.sync.dma_start(wk_sb[:], wk_d.rearrange("(c p) e -> p c e", p=128))
            apool_sb = initp.tile([128, 2, D], f32)
            nc.sync.dma_start(apool_sb[:], apool_d.rearrange("(c p) d -> p c d", p=128))
            mask_sb = constp.tile([128, 8, 64], f32)
            nc.sync.dma_start(mask_sb[:], mask_d)
            ident16 = constp.tile([128, 128], f16)
            nc.sync.dma_start(ident16[:], ident_d)
            bq_sb = constp.tile([E, 1], f32)
            bk_sb = constp.tile([E, 1], f32)
            b1_sb = constp.tile([H, 1], f32)
            b2_sb = constp.tile([1, 1], f32)
            for dst, src in ((bq_sb, bq_d), (bk_sb, bk_d), (b1_sb, b1_d), (b2_sb, b2_d)):
                nc.sync.dma_start(dst[:], src)

            # W1 pieces: rows [0:128]=q, [128:256]=k, [256:384]=d, [384:512]=p, [512]=prior
            w1_sb = initp.tile([128, 4, H], f32)
            nc.sync.dma_start(w1_sb[:], w1_d[0:512, :].rearrange("(c p) h -> p c h", p=128))
            w1pr_sb = constp.tile([1, H], f32)
            nc.sync.dma_start(w1pr_sb[:], w1_d[512:513, :])
            w1qp = constp.tile([128, H], f16)
            w1kp = constp.tile([128, H], f16)
            w1p = constp.tile([128, H], f16)
            w1pr16 = constp.tile([1, H], f16)
            nc.vector.tensor_add(w1qp[:], w1_sb[:, 0, :], w1_sb[:, 2, :])
            nc.vector.tensor_sub(w1kp[:], w1_sb[:, 1, :], w1_sb[:, 2, :])
            nc.vector.tensor_copy(w1p[:], w1_sb[:, 3, :])
            nc.vector.tensor_copy(w1pr16[:], w1pr_sb[:])
            w2_sb = initp.tile([H, 1], f32)
            nc.sync.dma_start(w2_sb[:], w2_d)
            w2_16 = constp.tile([H, 1], f16)
            # 0.5 factor of primitive gelu-tanh folded into W2 (prim path only)
            nc.vector.tensor_scalar_mul(w2_16[:], w2_sb[:], 1.0 if HW_GELU else 0.5)

            # Weff = A_pool^T @ W  -> stored as 8 chunks of (128 D-rows, E), fp16
            weffq = constp.tile([128, 8, E], f16)
            weffk = constp.tile([128, 8, E], f16)
            for wsb, weff in ((wq_sb, weffq), (wk_sb, weffk)):
                for r in range(8):
                    ps_w = psA.tile([128, 512], f32, tag="psA")
                    for k2 in range(2):
                        nc.tensor.matmul(ps_w[:, 0:E], apool_sb[:, k2, r * 128:(r + 1) * 128],
                                         wsb[:, k2, :], start=(k2 == 0), stop=(k2 == 1))
                    nc.scalar.activation(weff[:, r, :], ps_w[:, 0:E], AF.Copy)

            initp.release()
            encp = tc.alloc_tile_pool(name="enc", bufs=2)
            attnp = tc.alloc_tile_pool(name="attn_pool", bufs=3)
            smallp = tc.alloc_tile_pool(name="small", bufs=1)
            kpackp = tc.alloc_tile_pool(name="kpack", bufs=8)
            ccp = tc.alloc_tile_pool(name="cc", bufs=4)
            khfp = tc.alloc_tile_pool(name="khf_pool", bufs=4)
            outp = tc.alloc_tile_pool(name="outp", bufs=2)

            # DRAM scratch for kcat tables
            kcat_full = [dramp.tile([NK, E], f16, name=f"kcat_full{b}") for b in range(B)]

            # ============ encode both batches up front ============
            qts = []
            for b in range(B):
                bsb = encp.tile([128, 8, 512], f16, tag="encrhs")
                nc.sync.dma_start(bsb[:, :, 0:NQ], base_dm[b].rearrange("(c p) n -> p c n", p=128))
                ps_q = psA.tile([128, 512], f32, tag="psA")
                for k2 in range(8):
                    nc.tensor.matmul(ps_q[:, 0:NQ], weffq[:, k2, :], bsb[:, k2, 0:NQ],
                                     start=(k2 == 0), stop=(k2 == 7))
                qT16 = encp.tile([E, 512], f16, tag="qT16")
                nc.scalar.activation(qT16[:, 0:NQ], ps_q[:, 0:NQ], AF.Identity, bias=bq_sb[:, 0:1])
                ps_qp = psB.tile([128, 512], f32, tag="psB")
                nc.tensor.matmul(ps_qp[0:H, 0:NQ], w1qp[:], qT16[:, 0:NQ])
                qprojT = encp.tile([H, 512], f16, tag="qprojT")
                nc.scalar.activation(qprojT[:, 0:NQ], ps_qp[0:H, 0:NQ], AF.Copy)
                qts.append((qT16, qprojT))

                # keys: all NK encoded locally; kcat rows [emb|proj|pad] via PE transpose
                for kc in range(NK // 512):
                    ksb = encp.tile([128, 8, 512], f16, tag="encrhs")
                    nc.sync.dma_start(
                        ksb[:], hfk_dm[b, :, kc * 512:(kc + 1) * 512]
                        .rearrange("(c p) n -> p c n", p=128))
                    ps_k = psA.tile([128, 512], f32, tag="psA")
                    for k2 in range(8):
                        nc.tensor.matmul(ps_k[:], weffk[:, k2, :], ksb[:, k2, :],
                                         start=(k2 == 0), stop=(k2 == 7))
                    kT16 = encp.tile([E, 512], f16, tag="kT16")
                    nc.scalar.activation(kT16[:], ps_k[:], AF.Identity, bias=bk_sb[:, 0:1])
                    kcat_sb = smallp.tile([128, 4, E], f16, tag="kcat_sb", bufs=2)
                    for tt in range(4):
                        sl = slice(tt * 128, (tt + 1) * 128)
                        ps_t1 = psA.tile([128, 512], f16, tag="psA")
                        nc.tensor.transpose(ps_t1[:, 0:128], kT16[:, sl], ident16[:])
                        nc.scalar.activation(kcat_sb[:, tt, :], ps_t1[:, 0:128], AF.Copy)
                    nc.sync.dma_start(
                        kcat_full[b][kc * 512:(kc + 1) * 512, :]
                        .rearrange("(tt p) e -> p tt e", p=128),
                        kcat_sb[:])
            if dbg:
                kctmp = smallp.tile([128, NK // 128, E], f16, tag="kctmp", bufs=1)
                nc.sync.dma_start(kctmp[:], kcat_full[0][:].rearrange("(t p) e -> p t e", p=128))
                nc.sync.dma_start(dbg_kcat[:].rearrange("(t p) e -> p t e", p=128), kctmp[:])

            # ============ 3-stage software-pipelined tile loop ============
            tiles = [(b, t) for b in range(B) for t in range(cfg.nt)]
            st = {}

            def emit_topk(s):
                b, t = tiles[s]
                asb = attnp.tile([128, NK], f32, tag="attn_t", name=f"attn_{b}_{t}")
                nc.sync.dma_start(asb[:], attn[b, t * 128:(t + 1) * 128, :])
                idx_t = smallp.tile([128, K], u16, tag="idx_t", bufs=3, name=f"idx_{b}_{t}")
                prior_t = smallp.tile([128, K], f32, tag="prior_t", bufs=3, name=f"prior_{b}_{t}")
                nc.vector.max(prior_t[:, 0:8], asb[:])
                nc.vector.max_index(idx_t[:, 0:8], prior_t[:, 0:8], asb[:])
                nc.vector.match_replace(asb[:], prior_t[:, 0:8], asb[:], -1e30)
                nc.vector.max(prior_t[:, 8:16], asb[:])
                nc.vector.max_index(idx_t[:, 8:16], prior_t[:, 8:16], asb[:])
                pcl_t = smallp.tile([128, K], f32, tag="pcl_t", bufs=3, name=f"pcl_{b}_{t}")
                nc.vector.tensor_scalar_max(pcl_t[:], prior_t[:], 1e-8)
                if dbg and b == 0:
                    nc.sync.dma_start(dbg_idx[:, t, :], idx_t[:])
                    nc.sync.dma_start(dbg_prior[:, t, :], prior_t[:])
                idx_scr = dramp.tile([K, 128], u16, name=f"idx_scr{b}_{t}")
                nc.scalar.dma_start(idx_scr[:].rearrange("kk qq -> qq kk"), idx_t[:])
                pr_scr = dramp.tile([2048], f32, name=f"pr_scr{b}_{t}")
                nc.scalar.dma_start(
                    pr_scr[:].rearrange("(qq kk) -> qq kk", kk=K), prior_t[:])
                idxp1 = smallp.tile([128, 128], u16, tag="idxp1", bufs=3,
                                    name=f"idxp{b}_{t}")
                nc.scalar.dma_start(
                    idxp1[:],
                    idx_scr[:].unsqueeze(0).broadcast_to((8, K, 128)),
                )
                return dict(pcl_t=pcl_t, idxp1=idxp1, pr_scr=pr_scr)

            def emit_rescore(s):
                b, t = tiles[s]
                S = st[s]
                qT16, qprojT = qts[b]
                pcl_t, idxp1, pr_scr = S["pcl_t"], S["idxp1"], S["pr_scr"]
                resid_scr = dramp.tile([2048], f32, name=f"resid_scr{b}_{t}")
                priort_row = smallp.tile([1, 2048], f32, tag="priort_row", bufs=2,
                                         name=f"priorrow{b}_{t}")
                nc.scalar.dma_start(priort_row[:], pr_scr[:])
                for hh2 in range(4):
                    kpack = kpackp.tile([128, 1, 512], f16, tag="kpack")
                    nc.gpsimd.dma_gather(
                        kpack[:], kcat_full[b][:],
                        idxp1[:, hh2 * 32:(hh2 + 1) * 32].bitcast(i16),
                        512, 512, E, transpose=True,
                    )
                    if dbg and b == 0 and t == 0 and hh2 == 0:
                        nc.sync.dma_start(dbg_idxp[:, 0:128], idxp1[:])
                        nc.sync.dma_start(dbg_kpack[:, 0:1, 0:512], kpack[:])
                    nq0 = t * 128 + hh2 * 32
                    prod = ccp.tile([E, 512], f16, tag="prod")
                    nc.vector.tensor_mul(
                        prod[:].rearrange("p (n j) -> p n j", j=16),
                        kpack[:, 0, :].rearrange("p (n j) -> p n j", j=16),
                        qT16[:, nq0:nq0 + 32].unsqueeze(2).broadcast_to((E, 32, 16)),
                    )
                    ps_h = psA.tile([128, 512], f32, tag="psA")
                    nc.tensor.matmul(ps_h[0:H, :], w1p[:], prod[:], start=True, stop=False)
                    nc.tensor.matmul(ps_h[0:H, :], w1kp[:], kpack[:, 0, :],
                                     start=False, stop=False)
                    nc.tensor.matmul(ps_h[0:H, :], w1pr_sb[:],
                                     priort_row[:, hh2 * 512:(hh2 + 1) * 512],
                                     start=False, stop=True)
                    hin = ccp.tile([H, 512], f16, tag="hin")
                    nc.vector.scalar_tensor_tensor(
                        hin[:].rearrange("p (n j) -> p n j", j=16),
                        ps_h[0:H, :].rearrange("p (n j) -> p n j", j=16),
                        b1_sb[:, 0:1],
                        qprojT[:, nq0:nq0 + 32].unsqueeze(2).broadcast_to((H, 32, 16)),
                        ALU.add, ALU.add)
                    h16 = ccp.tile([H, 512], f16, tag="h16")
                    if HW_GELU:
                        nc.scalar.activation(h16[:], hin[:], AF.Gelu_apprx_tanh)
                    else:
                        t1 = ccp.tile([H, 512], f16, tag="t1")
                        nc.vector.tensor_mul(t1[:], hin[:], hin[:])
                        nc.vector.tensor_mul(t1[:], t1[:], hin[:])
                        nc.vector.scalar_tensor_tensor(t1[:], t1[:], 0.044715, hin[:],
                                                       ALU.mult, ALU.add)
                        th = ccp.tile([H, 512], f16, tag="th")
                        nc.scalar.activation(th[:], t1[:], AF.Tanh, scale=0.7978845608028654)
                        nc.vector.scalar_tensor_tensor(h16[:], th[:], 1.0, hin[:],
                                                       ALU.add, ALU.mult)
                    ps_r = psB.tile([128, 512], f32, tag="psB")
                    nc.tensor.matmul(ps_r[0:1, :], w2_16[:], h16[:])
                    residc = ccp.tile([1, 512], f32, tag="residc")
                    nc.vector.tensor_scalar_add(residc[:], ps_r[0:1, :], b2_sb[0:1, 0:1])
                    nc.scalar.dma_start(resid_scr[hh2 * 512:(hh2 + 1) * 512], residc[:])
                # softmax
                residq = smallp.tile([128, K], f32, tag="residq", bufs=2)
                nc.scalar.dma_start(
                    residq[:], resid_scr[:].rearrange("(qq kk) -> qq kk", kk=K))
                wexp = smallp.tile([128, K], f32, tag="wexp", bufs=2)
                nc.scalar.activation(wexp[:], residq[:], AF.Exp)
                wun = smallp.tile([128, K], f32, tag="wun", bufs=2)
                ssum = smallp.tile([128, 1], f32, tag="ssum", bufs=2)
                nc.vector.scalar_tensor_tensor(wun[:], wexp[:], 1.0, pcl_t[:],
                                               ALU.mult, ALU.mult, accum_out=ssum[:])
                rs = smallp.tile([128, 1], f32, tag="rs", bufs=2)
                nc.vector.reciprocal(rs[:], ssum[:])
                wnorm = smallp.tile([128, K], f32, tag="wnorm", bufs=2)
                nc.vector.tensor_tensor(wnorm[:], wun[:],
                                        rs[:].broadcast_to((128, K)), ALU.mult)
                wn_scr = dramp.tile([2048], f32, name=f"wn_scr{b}_{t}")
                nc.scalar.dma_start(
                    wn_scr[:].rearrange("(qq kk) -> qq kk", kk=K), wnorm[:])
                wpair = smallp.tile([128, 16, 1], f32, tag="wpair", bufs=2)
                nc.scalar.dma_start(
                    wpair[:, :, 0], wn_scr[:].rearrange("(blk p) -> p blk", p=128))
                wblk_t = smallp.tile([128, 16, 64], f16, tag="wblk", bufs=2,
                                     name=f"wblk{b}_{t}")
                nc.vector.scalar_tensor_tensor(
                    wblk_t[:].rearrange("p (gm j) q -> p gm j q", j=8),
                    wpair[:].rearrange("p (gm j) one -> p gm j one", j=8)
                        .broadcast_to((128, 2, 8, 64)),
                    1.0,
                    mask_sb[:].unsqueeze(1).broadcast_to((128, 2, 8, 64)),
                    ALU.mult, ALU.mult,
                )
                if dbg and b == 0:
                    rtmp = smallp.tile([128, 16], f32, tag="rtmp", bufs=1)
                    nc.sync.dma_start(rtmp[:], resid_scr[:].rearrange("(blk p) -> p blk", p=128))
                    nc.sync.dma_start(dbg_resid[t * 2048:(t + 1) * 2048].rearrange("(blk p) -> p blk", p=128), rtmp[:])
                    wtmp = smallp.tile([128, 16], f32, tag="wtmp", bufs=1)
                    nc.sync.dma_start(wtmp[:], wn_scr[:].rearrange("(blk p) -> p blk", p=128))
                    nc.sync.dma_start(dbg_wn[t * 2048:(t + 1) * 2048].rearrange("(blk p) -> p blk", p=128), wtmp[:])
                    nc.sync.dma_start(dbg_wblk[:, t * 16:(t + 1) * 16, :], wblk_t[:])
                S["wblk_t"] = wblk_t

            def emit_wsum(s):
                b, t = tiles[s]
                S = st[s]
                idxp1, wblk_t = S["idxp1"], S["wblk_t"]
                ps_o = psO.tile([128, D], f32, tag="psO")
                for g2 in range(2):
                    khf = khfp.tile([128, 8, D], f16, tag="khf")
                    nc.gpsimd.dma_gather(
                        khf[:], hf16[b],
                        idxp1[:, g2 * 64:(g2 + 1) * 64].bitcast(i16),
                        1024, 1024, D, transpose=False,
                    )
                    if dbg and b == 0 and t == 0 and g2 == 0:
                        nc.sync.dma_start(dbg_khf[:], khf[:])
                    base = 64 * g2
                    for csl in (slice(0, 512), slice(512, D)):
                        for j in range(8):
                            nc.tensor.matmul(
                                ps_o[base:base + 64, csl],
                                wblk_t[:, g2 * 8 + j, :],
                                khf[:, j, csl],
                                start=(j == 0), stop=(j == 7),
                            )
                osb = outp.tile([128, D], f32, tag="osb")
                nc.scalar.activation(osb[:], ps_o[:], AF.Copy)
                nc.sync.dma_start(out_d[b, t * 128:(t + 1) * 128, :], osb[:])

            NTILES = len(tiles)
            for s in range(NTILES + 2):
                if s < NTILES:
                    st[s] = emit_topk(s)
                if 1 <= s <= NTILES:
                    emit_rescore(s - 1)
                if s >= 2:
                    emit_wsum(s - 2)

            for p_ in (outp, khfp, ccp, kpackp, smallp, attnp, encp):
                p_.release()

    nc.compile()
    return nc


# ---------------------------------------------------------------------------
# Host side
# ---------------------------------------------------------------------------

def _make_apool():
    A = np.zeros((STEM_C * POOL * POOL, STEM_C * P * P), np.float32)
    s = P // POOL
    for c in range(STEM_C):
        for py in range(POOL):
            for px in range(POOL):
                o = (c * POOL + py) * POOL + px
                for dy in range(s):
                    for dx in range(s):
                        d = (c * P + py * s + dy) * P + px * s + dx
                        A[o, d] = 1.0 / (s * s)
    return A


def make_in_maps(inputs, cfg: Cfg):
    B, D = cfg.B, cfg.D
    NQ, NK, NC = cfg.nq, cfg.nk, cfg.ncores
    hr_attn = np.asarray(inputs["hr_attn"], np.float32)
    hr_hf = np.asarray(inputs["hr_hf_patches"], np.float32).reshape(B, D, NK)
    base_hf = np.asarray(inputs["base_hf_patches"], np.float32).reshape(B, D, NK)
    hf16 = np.ascontiguousarray(hr_hf.transpose(0, 2, 1)).astype(np.float16)

    common = dict(
        wq=np.asarray(inputs["Wq"], np.float32),
        wk=np.asarray(inputs["Wk"], np.float32),
        w1=np.asarray(inputs["W1"], np.float32),
        w2=np.asarray(inputs["W2"], np.float32).reshape(cfg.H, 1),
        bq=np.asarray(inputs["bq"], np.float32).reshape(cfg.E, 1),
        bk=np.asarray(inputs["bk"], np.float32).reshape(cfg.E, 1),
        b1=np.asarray(inputs["b1"], np.float32).reshape(cfg.H, 1),
        b2=np.asarray(inputs["b2"], np.float32).reshape(1, 1),
        apool=_make_apool(),
        maskblk=np.equal(np.arange(64)[None, None, :], 8 * np.arange(8)[None, :, None] + (np.arange(128) // 16)[:, None, None]).astype(np.float32),
        ident16=np.eye(128, dtype=np.float16),
        hf16=hf16,
        hf_dm16=hr_hf.astype(np.float16),
    )
    in_maps = []
    for c in range(NC):
        sl = slice(c * NQ, (c + 1) * NQ)
        m = dict(common)
        m["attn"] = np.ascontiguousarray(hr_attn[:, sl, :])
        m["base_dm16"] = np.ascontiguousarray(base_hf[:, :, sl]).astype(np.float16)
        in_maps.append(m)
    return in_maps


_NC_CACHE = {}


def _get_nc(cfg: Cfg):
    key = (cfg.nq, cfg.nk, cfg.ncores)
    if key not in _NC_CACHE:
        _NC_CACHE[key] = build_nc(cfg)
    return _NC_CACHE[key]


def run(inputs, trace=False, cfg=None, dbg=False):
    cfg = cfg or Cfg()
    if dbg:
        nc = build_nc(cfg, dbg=True)
    else:
        nc = _get_nc(cfg)
    in_maps = make_in_maps(inputs, cfg)
    res = run_bass_kernel_spmd(nc, in_maps, core_ids=list(range(cfg.ncores)),
                               trace=trace)
    B, D, NQ, NC = cfg.B, cfg.D, cfg.nq, cfg.ncores
    out = np.empty((B, NC * NQ, D), np.float32)
    for c in range(NC):
        out[:, c * NQ:(c + 1) * NQ, :] = res.results[c]["out"]
    return out, res


def kernel(**inputs) -> np.ndarray:
    tk = inputs.get("topk", 16)
    assert int(np.asarray(tk)) == 16, "kernel is specialized for topk=16"
    out, res = run(inputs, trace=bool(os.environ.get("BASS_KERNEL_TRACE")))
    if res.exec_time_ns is not None:
        print(f"HW exec time: {res.exec_time_ns} ns")
    return out

